# revision 4
# baseline (speedup 1.0000x reference)
"""Multi-head attention (B=2, L=2048, D=1024, H=16, hd=64) on 8 TRN2 NeuronCores.

Sharding: tensor-parallel over heads — 2 heads per core. Each core computes
qkv projection for its heads, full attention for its (b, h) pairs, and a
partial output projection (w_proj columns for its heads); the host sums the
8 partial projections.

All matmuls run in float32r (tf32) at full PE rate; inputs are pre-rounded
to tf32 on the host so operand rounding is exact. PSUM accumulation is fp32.

Dataflow per core (all layouts keep the contraction dim on partitions):
  qT,kT,vT [128, 4096] = w-slice.T @ xT        (transposed layout [j, t])
  v re-transposed to [tk, dh] via PE transpose, augmented with a ones column
  scoresT  [tk, tq] = kT.T-slices @ qT-slices  (2 heads packed via tile_position)
  expT = exp(scoresT)                          (ACT, psum->sbuf, 2-bank reads)
  outU [65, tq] = [v|1].T @ expT               (fused attn@v + softmax denominator)
  head = outU[0:64] * broadcast(1/outU[64])    (DVE recip + GPSIMD partition bcast)
  outT_partial [e, t] = wprojT-slices.T @ headT
"""
import sys

if '/opt/trn_rl_repo' not in sys.path:
    sys.path.insert(0, '/opt/trn_rl_repo')

import numpy as np

B, L, D = 2, 2048, 1024
HEAD_DIM = 64
H = D // HEAD_DIM          # 16
NCORES = 8
HPC = H // NCORES          # 2 heads per core
T = B * L                  # 4096
KT = D // 128              # 8 contraction tiles for the projections
TT = T // 512              # 8 t-tiles of 512
TQ = L // 512              # 4 query tiles per batch
TK = L // 128              # 16 key tiles per batch


def tf32_round(x: np.ndarray) -> np.ndarray:
    xi = np.ascontiguousarray(x, dtype=np.float32).view(np.uint32)
    return ((xi + 0x1000) & 0xFFFFE000).view(np.float32)


def _build_nc(reps: int = 1):
    import concourse.bacc as bacc
    import concourse.mybir as mybir
    import concourse.tile as tile
    from concourse.masks import make_identity

    F32 = mybir.dt.float32
    F32R = mybir.dt.float32r
    EXP = mybir.ActivationFunctionType.Exp

    nc = bacc.Bacc("TRN2", target_bir_lowering=False, debug=False,
                   num_devices=NCORES)
    xT_d = nc.dram_tensor("xT", [D, T], F32R, kind="ExternalInput").ap()
    wqkvT_d = nc.dram_tensor("wqkvT", [D, 3 * 128], F32R, kind="ExternalInput").ap()
    wprojT_d = nc.dram_tensor("wprojT", [128, D], F32R, kind="ExternalInput").ap()
    outT_d = nc.dram_tensor("outT", [D, T], F32, kind="ExternalOutput").ap()

    with tile.TileContext(nc) as tc:
        with nc.allow_low_precision(reason="tf32 matmul pipeline by design"), \
             tc.tile_pool(name="const", bufs=1) as cp, \
             tc.tile_pool(name="xt", bufs=2) as xp, \
             tc.tile_pool(name="exp", bufs=4) as ep, \
             tc.tile_pool(name="nrm", bufs=2) as np_, \
             tc.tile_pool(name="ps", bufs=2, space="PSUM") as ps:

            # constants
            ident_f = cp.tile([128, 128], F32, tag="identf")
            make_identity(nc, ident_f[:])
            ident = cp.tile([128, 128], F32R, tag="ident")
            nc.vector.tensor_copy(ident[:], ident_f[:])
            ones_f = cp.tile([128, 1], F32, tag="onesf")
            nc.gpsimd.memset(ones_f[:], 1.0)

            # weights
            w_t = [cp.tile([128, 384], F32R, tag=f"w{k}", name=f"w{k}")
                   for k in range(KT)]
            for k in range(KT):
                nc.sync.dma_start(w_t[k][:], wqkvT_d[k * 128:(k + 1) * 128, :])
            wp_t = cp.tile([128, 1024], F32R, tag="wp")
            nc.sync.dma_start(wp_t[:], wprojT_d[:, :])

            # persistent activations
            qT = cp.tile([128, T], F32R, tag="qT")
            kTt = cp.tile([128, T], F32R, tag="kTt")
            vT = cp.tile([128, T], F32R, tag="vT")
            headT = cp.tile([128, T], F32R, tag="headT")
            vblk = [[cp.tile([128, 132], F32R, tag=f"vb{b}_{tk}",
                             name=f"vb{b}_{tk}") for tk in range(TK)]
                    for b in range(B)]

            from contextlib import nullcontext
            with (tc.For_i(0, reps, 1) if reps > 1 else nullcontext()):
                for b in range(B):
                    # ---- stage A: qkv projection for this batch's t-tiles ----
                    for tt in range(TT // B):
                        t = b * (TT // B) + tt
                        t0 = t * 512
                        xt = [xp.tile([128, 512], F32R, tag=f"xt{k}",
                                      name=f"xt{k}") for k in range(KT)]
                        for k in range(KT):
                            nc.sync.dma_start(xt[k][:], xT_d[k * 128:(k + 1) * 128,
                                                             t0:t0 + 512])
                        for part, dest in ((0, qT), (1, kTt), (2, vT)):
                            s = ps.tile([128, 1024], F32, tag="sc", name="sA")
                            for k in range(KT):
                                nc.tensor.matmul(
                                    s[:, 0:512],
                                    w_t[k][:, part * 128:(part + 1) * 128],
                                    xt[k][:],
                                    start=(k == 0), stop=(k == KT - 1))
                            nc.vector.tensor_copy(dest[:, t0:t0 + 512], s[:, 0:512])

                    # ---- v transpose into [tk, dh] blocks with ones columns ----
                    for tk in range(TK):
                        c0 = b * L + tk * 128
                        p = ps.tile([128, 512], F32R, tag="pj", name="ptr")
                        nc.tensor.transpose(p[:, 0:128], vT[:, c0:c0 + 128], ident[:])
                        vb = vblk[b][tk]
                        nc.vector.tensor_copy(vb[:, 0:64], p[0:128, 0:64])
                        nc.vector.tensor_copy(vb[:, 66:130], p[0:128, 64:128])
                        nc.vector.tensor_copy(vb[:, 64:65], ones_f[:])
                        nc.vector.tensor_copy(vb[:, 130:131], ones_f[:])

                    # ---- stage B: attention for (b, h0), (b, h1) ----
                    for tq in range(TQ):
                        q0 = b * L + tq * 512
                        ou = [ps.tile([65, 512], F32, tag="outU", name=f"ou{h}")
                              for h in range(2)]
                        for tk in range(TK):
                            k0 = b * L + tk * 128
                            s = ps.tile([128, 1024], F32, tag="sc", name="sB")
                            nc.tensor.matmul(s[:, 0:512],
                                             kTt[0:64, k0:k0 + 128],
                                             qT[0:64, q0:q0 + 512],
                                             start=True, stop=True,
                                             tile_position=(0, 0))
                            nc.tensor.matmul(s[:, 512:1024],
                                             kTt[64:128, k0:k0 + 128],
                                             qT[64:128, q0:q0 + 512],
                                             start=True, stop=True,
                                             tile_position=(64, 0))
                            e = ep.tile([128, 1024], F32R, tag="e", name="e")
                            nc.scalar.activation(e[:], s[:], EXP)
                            nc.tensor.matmul(ou[0][:], vblk[b][tk][:, 0:65],
                                             e[:, 0:512],
                                             start=(tk == 0), stop=(tk == TK - 1))
                            nc.tensor.matmul(ou[1][:], vblk[b][tk][:, 66:131],
                                             e[:, 512:1024],
                                             start=(tk == 0), stop=(tk == TK - 1))
                        for h in range(2):
                            su = np_.tile([64, 512], F32, tag="su", name="su")
                            nc.vector.tensor_copy(su[:], ou[h][0:64, :])
                            r = np_.tile([1, 512], F32, tag="r", name="r")
                            nc.vector.reciprocal(r[:], ou[h][64:65, :])
                            bc = np_.tile([64, 512], F32, tag="bc", name="bc")
                            nc.gpsimd.partition_broadcast(bc[:], r[:])
                            nc.vector.tensor_mul(
                                headT[h * 64:(h + 1) * 64, q0:q0 + 512],
                                su[:], bc[:])

                    # ---- proj partial for this batch's t-tiles ----
                    for tt in range(TT // B):
                        t0 = (b * (TT // B) + tt) * 512
                        for e8 in range(8):
                            pp = ps.tile([128, 512], F32, tag="pj", name="pp")
                            nc.tensor.matmul(pp[:],
                                             wp_t[:, e8 * 128:(e8 + 1) * 128],
                                             headT[:, t0:t0 + 512],
                                             start=True, stop=True)
                            po = xp.tile([128, 512], F32, tag="po", name="po",
                                         bufs=3)
                            nc.vector.tensor_copy(po[:], pp[:])
                            nc.sync.dma_start(
                                outT_d[e8 * 128:(e8 + 1) * 128, t0:t0 + 512],
                                po[:])

    nc.compile()
    return nc


_CACHE = {}


def _get_nc(reps: int = 1):
    if reps not in _CACHE:
        _CACHE[reps] = _build_nc(reps)
    return _CACHE[reps]


def _make_in_maps(x, w_qkv, w_proj):
    xT = tf32_round(np.ascontiguousarray(x.reshape(T, D).T))
    in_maps = []
    for c in range(NCORES):
        j0 = c * 128
        wq = w_qkv[j0:j0 + 128] * 0.125          # fold attention scale into q
        wk = w_qkv[D + j0:D + j0 + 128]
        wv = w_qkv[2 * D + j0:2 * D + j0 + 128]
        wqkvT = tf32_round(np.ascontiguousarray(
            np.concatenate([wq, wk, wv], axis=0).T))
        wprojT = tf32_round(np.ascontiguousarray(w_proj[:, j0:j0 + 128].T))
        in_maps.append({"xT": xT, "wqkvT": wqkvT, "wprojT": wprojT})
    return in_maps


def _numpy_reference(x, mask, w_qkv, w_proj):
    x64 = x.astype(np.float64)
    qkv = (x64 @ w_qkv.T.astype(np.float64)).reshape(B, L, 3, H, HEAD_DIM)
    qkv = qkv.transpose(2, 0, 3, 1, 4)
    q, k, v = qkv[0], qkv[1], qkv[2]
    attn = np.einsum('bhqd,bhkd->bhqk', q, k) * (HEAD_DIM ** -0.5)
    attn = np.where(mask[:, None, :, :], attn, -np.inf)
    attn = attn - attn.max(axis=-1, keepdims=True)
    attn = np.exp(attn)
    attn = attn / attn.sum(axis=-1, keepdims=True)
    out = np.einsum('bhqk,bhkd->bhqd', attn, v)
    out = out.transpose(0, 2, 1, 3).reshape(B, L, D)
    return (out @ w_proj.T.astype(np.float64)).astype(np.float32)


def kernel(x, mask, w_qkv, w_proj):
    x = np.asarray(x)
    mask = np.asarray(mask)
    w_qkv = np.asarray(w_qkv)
    w_proj = np.asarray(w_proj)
    if not mask.all():
        # spec guarantees an all-ones mask; keep a correct fallback anyway
        return _numpy_reference(x, mask, w_qkv, w_proj)

    from concourse import bass_utils
    nc = _get_nc()
    in_maps = _make_in_maps(x, w_qkv, w_proj)
    res = bass_utils.run_bass_kernel_spmd(nc, in_maps,
                                          core_ids=list(range(NCORES)))
    acc = np.zeros((D, T), np.float32)
    for c in range(NCORES):
        acc += res.results[c]["outT"]
    return np.ascontiguousarray(acc.T).reshape(B, L, D)


if __name__ == "__main__":
    rng = np.random.default_rng(0)
    x = rng.standard_normal((B, L, D)).astype(np.float32)
    mask = np.ones((B, L, L), bool)
    w_qkv = (rng.standard_normal((3 * D, D)) * D ** -0.5).astype(np.float32)
    w_proj = (rng.standard_normal((D, D)) * D ** -0.5).astype(np.float32)
    out = kernel(x, mask, w_qkv, w_proj)
    exp = _numpy_reference(x, mask, w_qkv, w_proj)
    err = np.abs(out - exp).max() / np.abs(exp).max()
    print("rel err vs fp64 numpy reference:", err)
